# revision 8
# baseline (speedup 1.0000x reference)
"""Causal self-attention (q/k swapped variant) on 8 TRN2 NeuronCores.

Data-parallel over batch: core b computes the full transformer block for
x[b]. Host-side prep transposes x and the four weight matrices so every
on-chip matmul sees its contraction dim on SBUF partitions with natural
(contiguous) DMA layouts — no on-chip transposes anywhere.

Math (per core, T=1024, C=1024, H=16, D=64):
  qT = (x Wq^T)^T, kT = (x Wk^T)^T   stored [o, t]
  v  = x Wv^T                         stored [t, o] (+ ones column per head)
  S^T[j, i] = sum_d q[j,d] k[i,d]     (reference: att[i,j] = k_i . q_j)
  A = exp(S^T / 8) * causal_mask      (no max-subtraction: |logits| < ~4)
  Y'[d+, i] = V'^T A  with V' = [v_h | 1]  -> row D holds softmax denom
  yT[o, i] = Y'[0:D] * (1/Y'[D]) broadcast (K=1 matmul broadcast trick)
  out = yT^T Wp^T + bp                (bias via K=1 ones x bp matmul)
"""

import numpy as np

# ---------------------------------------------------------------------------
# Workaround for walrus "Too many sync wait commands" on the TileContext
# kernel-tail Drain: this walrus build accepts only 1 sem wait per CTRL
# instruction. Chunk the global-clock waits onto SP nops (1 each) so the
# Drain itself needs none.
# ---------------------------------------------------------------------------
import concourse.tile as tile
from concourse.vector_clock import ScopedClock


def _patched_drain_and_barrier(self, tick_clock, wait_clock):
    vec = tick_clock.global_clock
    entries = [(p, vec[p]) for p in range(len(vec)) if vec[p] > 0]
    for p, t in entries:
        nop = self.nc.sync.nop(nofuse=True, hint="drain_wait_chunk")
        part = ScopedClock()
        part.require_at_least(None, p, t)
        wait_clock.add_sem_waits(nop.ins, part)
    # SP nops above waited out the full clock in program order on the same
    # queue, so the drain carries no waits of its own.
    self.nc.sync.drain()
    self.nc.all_engine_barrier()
    assert self.sems is not None
    popped = self.nc._tile_sem_poison_stack.pop()
    assert popped is self._sem_poison
    self.nc.clear_and_free_semaphores(list(self.sems.allocated().values()))
    self.nc.all_engine_barrier()


tile.TileContext._drain_and_barrier = _patched_drain_and_barrier

import concourse.bass as bass
import concourse.mybir as mybir
from concourse.bass_utils import run_bass_kernel_spmd

F32 = mybir.dt.float32
BF16 = mybir.dt.bfloat16

B, T, C = 8, 1024, 1024
H, D = 16, 64
P = 128
F = 512
NT = T // P  # 8  (128-chunks of t / j)
NCC = C // P  # 8  (128-chunks of c / o)
N2 = T // F  # 2  (512-chunks of t / i / m)
NCORES = 8

# matmul compute dtype for projections / c_proj ("f32r", "f32", "bf16")
PROJ_MODE = "f32r"
# attention storage+matmul dtype for qT/kT/v/A/masks ("f32", "bf16")
ATT_MODE = "f32"


def _mm_cast(ap, mode):
    if mode == "f32r":
        return ap.bitcast(mybir.dt.float32r)
    return ap


def _split_multi_waits(nc):
    """This walrus build accepts only one sem wait per instruction. Hoist
    extra waits onto same-engine NoOps emitted just before the instruction
    (engine queues execute in program order, so semantics are preserved)."""
    for f in nc.m.functions:
        for bb in f.blocks:
            out = []
            changed = False
            for ins in bb.instructions:
                si = ins.sync_info
                if si is not None and si.on_wait and len(si.on_wait) > 1:
                    waits = list(si.on_wait)
                    for w in waits[:-1]:
                        nop = mybir.InstNoOp(
                            name=nc.get_next_instruction_name(),
                            engine=ins.engine,
                            ins=[],
                            outs=[],
                            sync_info=mybir.SyncInfo(on_wait=[w], on_update=[]),
                            text_hint="wait_split",
                        )
                        out.append(nop)
                    ins.sync_info = mybir.SyncInfo(
                        on_wait=[waits[-1]], on_update=si.on_update
                    )
                    changed = True
                out.append(ins)
            if changed:
                bb.instructions = out


def build_nc(proj_mode=PROJ_MODE, att_mode=ATT_MODE, trace_scopes=False):
    pdt = mybir.dt.float32r if proj_mode == "f32r" else F32
    att_dt = BF16 if att_mode == "bf16" else pdt
    nc = bass.Bass()

    xT_d = nc.dram_tensor("xT", [C, T], pdt, kind="ExternalInput")
    wq_d = nc.dram_tensor("WqT", [C, C], pdt, kind="ExternalInput")
    wk_d = nc.dram_tensor("WkT", [C, C], pdt, kind="ExternalInput")
    wv_d = nc.dram_tensor("WvT", [C, C], pdt, kind="ExternalInput")
    wp_d = nc.dram_tensor("WpT", [C, C], pdt, kind="ExternalInput")
    bp_d = nc.dram_tensor("bp", [1, C], pdt, kind="ExternalInput")
    mask_d = nc.dram_tensor("masks", [P, 4 * F], att_dt, kind="ExternalInput")
    out_d = nc.dram_tensor("out", [T, C], F32, kind="ExternalOutput")

    with tile.TileContext(nc) as tc:
        with (
            tc.tile_pool(name="big", bufs=1) as big,
            tc.tile_pool(name="w", bufs=8) as wpool,
            tc.tile_pool(name="a", bufs=3) as apool,
            tc.tile_pool(name="const", bufs=1) as cpool,
            tc.tile_pool(name="small", bufs=2) as small,
            tc.tile_pool(name="pmm", bufs=2, space="PSUM") as pmm,
            tc.tile_pool(name="pS", bufs=2, space="PSUM") as pS,
            tc.tile_pool(name="pY", bufs=2, space="PSUM") as pY,
        ):
            # ---- constants & resident inputs ----
            X = big.tile([P, NCC, T], pdt, tag="bigx")
            nc.sync.dma_start(X[:], xT_d.rearrange("(cc p) t -> p cc t", p=P))
            masks = cpool.tile([P, 4, F], att_dt)
            nc.sync.dma_start(masks[:], mask_d.rearrange("p (m f) -> p m f", f=F))
            bp_sb = cpool.tile([1, C], pdt)
            nc.sync.dma_start(bp_sb[:], bp_d[:])
            # memset cannot write float32r on this toolchain: stage ones in
            # fp32, then convert via tensor_copy.
            ones_f = cpool.tile([P, P], F32)
            nc.vector.memset(ones_f[:], 1.0)
            ones_t = cpool.tile([P, P], pdt)
            nc.vector.tensor_copy(out=ones_t[:], in_=ones_f[:])

            qT = big.tile([P, NCC, T], att_dt, tag="qT")
            kT = big.tile([P, NCC, T], att_dt, tag="kT")
            v = big.tile([P, NT, H, D + 1], att_dt, tag="v")
            # ones-augmentation column per (t-chunk, head)
            for t8 in range(NT):
                nc.vector.tensor_copy(out=v[:, t8, :, D], in_=ones_f[:, 0:H])

            def load_w(wd):
                tiles = []
                for cc in range(NCC):
                    t_ = wpool.tile([P, C], pdt, tag="w")
                    nc.sync.dma_start(t_[:], wd[cc * P : (cc + 1) * P, :])
                    tiles.append(t_)
                return tiles

            def pcast(ap):
                return ap

            def acast(ap):
                return ap

            # ---- v projection: v[t, o] = x Wv^T ----
            wv = load_w(wv_d)
            for on2 in range(N2):
                for t8 in range(NT):
                    ps = pmm.tile([P, F], F32, tag="mm")
                    for cc in range(NCC):
                        nc.tensor.matmul(
                            ps[:],
                            pcast(X[:, cc, t8 * P : (t8 + 1) * P]),
                            pcast(wv[cc][:, on2 * F : (on2 + 1) * F]),
                            start=(cc == 0),
                            stop=(cc == NCC - 1),
                        )
                    nc.vector.tensor_copy(
                        out=v[:, t8, on2 * 8 : (on2 + 1) * 8, 0:D],
                        in_=ps.rearrange("p (h d) -> p h d", d=D),
                    )

            # ---- q/k projections, transposed out: [o, t] ----
            for wd, dst in ((wq_d, qT), (wk_d, kT)):
                wt = load_w(wd)
                for oc in range(NCC):
                    for t2 in range(N2):
                        ps = pmm.tile([P, F], F32, tag="mm")
                        for cc in range(NCC):
                            nc.tensor.matmul(
                                ps[:],
                                pcast(wt[cc][:, oc * P : (oc + 1) * P]),
                                pcast(X[:, cc, t2 * F : (t2 + 1) * F]),
                                start=(cc == 0),
                                stop=(cc == NCC - 1),
                            )
                        nc.vector.tensor_copy(
                            out=dst[:, oc, t2 * F : (t2 + 1) * F], in_=ps[:]
                        )

            # ---- attention (heads x i-chunks) ----
            # yT shares the X slot: X's last readers are the projections.
            yT = big.tile([P, NCC, T], pdt, tag="bigx")
            for h in range(H):
                p0 = 64 * (h % 2)
                oc = h // 2
                for ic in range(N2):
                    jcs = [jc for jc in range(NT) if jc * P <= ic * F + F - 1]
                    pairs = [jcs[i : i + 2] for i in range(0, len(jcs), 2)]
                    Y = pY.tile([P, F], F32, tag="Y")
                    a_tiles = []
                    for pair in pairs:
                        Sp = pS.tile([P, 2 * F], F32, tag="S")
                        for xi, jc in enumerate(pair):
                            nc.tensor.matmul(
                                Sp[:, xi * F : (xi + 1) * F],
                                acast(qT[p0 : p0 + 64, oc, jc * P : (jc + 1) * P]),
                                acast(kT[p0 : p0 + 64, oc, ic * F : (ic + 1) * F]),
                                start=True,
                                stop=True,
                            )
                        A = apool.tile([P, 2 * F], att_dt, tag="A")
                        nc.scalar.activation(
                            out=A[:, 0 : len(pair) * F],
                            in_=Sp[:, 0 : len(pair) * F],
                            func=mybir.ActivationFunctionType.Exp,
                            scale=float(D) ** -0.5,
                        )
                        for xi, jc in enumerate(pair):
                            delta = jc * P - ic * F
                            if delta >= 0:  # partial (diagonal-band) block
                                mi = delta // P
                                nc.vector.tensor_mul(
                                    out=A[:, xi * F : (xi + 1) * F],
                                    in0=A[:, xi * F : (xi + 1) * F],
                                    in1=masks[:, mi, :],
                                )
                        a_tiles.append((A, pair))
                    n_mm = len(jcs)
                    cnt = 0
                    for A, pair in a_tiles:
                        for xi, jc in enumerate(pair):
                            nc.tensor.matmul(
                                Y[0 : D + 1, :],
                                acast(v[:, jc, h, :]),
                                acast(A[:, xi * F : (xi + 1) * F]),
                                start=(cnt == 0),
                                stop=(cnt == n_mm - 1),
                            )
                            cnt += 1
                    # softmax denominator -> reciprocal -> broadcast matmul
                    r = small.tile([D + 1, F], pdt, tag="r")
                    with nc.allow_low_precision(
                        reason="float32r out is bit-identical storage to fp32"
                    ):
                        nc.vector.reciprocal(
                            out=r[D : D + 1, :], in_=Y[D : D + 1, :]
                        )
                    Bc = pmm.tile([64, F], F32, tag="mm")
                    nc.tensor.matmul(
                        Bc[:],
                        ones_t[D : D + 1, 0:64],
                        r[D : D + 1, :],
                        start=True,
                        stop=True,
                    )
                    Bs = small.tile([64, F], F32, tag="Bs")
                    nc.vector.tensor_copy(out=Bs[:], in_=Bc[:])
                    if h % 2 == 0:
                        nc.vector.tensor_mul(
                            out=yT[0:64, oc, ic * F : (ic + 1) * F],
                            in0=Y[0:D, :],
                            in1=Bs[:],
                        )
                    else:
                        tmp = small.tile([64, F], pdt, tag="tmp")
                        nc.vector.tensor_mul(out=tmp[:], in0=Y[0:D, :], in1=Bs[:])
                        nc.sync.dma_start(
                            yT[64:128, oc, ic * F : (ic + 1) * F], tmp[:]
                        )

            # ---- c_proj: out = yT^T Wp^T + bp ----
            wp = load_w(wp_d)
            for t8 in range(NT):
                for mc in range(N2):
                    ps = pmm.tile([P, F], F32, tag="mm")
                    for oc in range(NCC):
                        nc.tensor.matmul(
                            ps[:],
                            pcast(yT[:, oc, t8 * P : (t8 + 1) * P]),
                            pcast(wp[oc][:, mc * F : (mc + 1) * F]),
                            start=(oc == 0),
                            stop=False,
                        )
                    nc.tensor.matmul(
                        ps[:],
                        ones_t[0:1, 0:P],
                        bp_sb[0:1, mc * F : (mc + 1) * F],
                        start=False,
                        stop=True,
                    )
                    zs = small.tile([P, F], F32, tag="z")
                    nc.vector.tensor_copy(out=zs[:], in_=ps[:])
                    nc.sync.dma_start(
                        out_d[t8 * P : (t8 + 1) * P, mc * F : (mc + 1) * F], zs[:]
                    )
    _split_multi_waits(nc)
    return nc


def _build_masks(att_mode=ATT_MODE):
    # mask[p, mi, f] = 1 if f >= p + mi*128 else 0  (keep i >= j)
    p = np.arange(P)[:, None, None]
    mi = np.arange(4)[None, :, None]
    f = np.arange(F)[None, None, :]
    m = (f >= p + mi * P).astype(np.float32)
    if att_mode == "bf16":
        import ml_dtypes

        m = m.astype(ml_dtypes.bfloat16)
    return np.ascontiguousarray(m.reshape(P, 4 * F))


_NC_CACHE = {}


def kernel(x, Wk, Wq, Wv, Wp, bp, _trace=False):
    x = np.ascontiguousarray(np.asarray(x, dtype=np.float32))
    WqT = np.ascontiguousarray(np.asarray(Wq, dtype=np.float32).T)
    WkT = np.ascontiguousarray(np.asarray(Wk, dtype=np.float32).T)
    WvT = np.ascontiguousarray(np.asarray(Wv, dtype=np.float32).T)
    WpT = np.ascontiguousarray(np.asarray(Wp, dtype=np.float32).T)
    bp2 = np.ascontiguousarray(np.asarray(bp, dtype=np.float32).reshape(1, C))
    xT = np.ascontiguousarray(np.transpose(x, (0, 2, 1)))  # [B, C, T]
    masks = _build_masks()

    key = (PROJ_MODE, ATT_MODE)
    if key not in _NC_CACHE:
        _NC_CACHE[key] = build_nc(*key)
    nc = _NC_CACHE[key]

    in_maps = [
        {
            "xT": xT[b],
            "WqT": WqT,
            "WkT": WkT,
            "WvT": WvT,
            "WpT": WpT,
            "bp": bp2,
            "masks": masks,
        }
        for b in range(NCORES)
    ]
    res = run_bass_kernel_spmd(
        nc, in_maps, core_ids=list(range(NCORES)), trace=_trace
    )
    out = np.stack([res.results[b]["out"] for b in range(NCORES)], axis=0)
    if _trace:
        kernel.last_result = res
    return out
